# revision 1
# baseline (speedup 1.0000x reference)
# Distributed kNN-retrieval loss kernel for Trainium2 (8 NeuronCores).
#
# Reference computation: two linear heads + softmax, a feature bank updated at
# trg_idx rows (no-grad), cosine kNN against the bank, KL pseudo-label loss +
# entropy/IM + label-smoothed CE. Output: scalar loss.
#
# Strategy (all compute on-device; host only shards/lays out inputs):
#  * fea_bank sharded row-wise across 8 cores, shipped pre-transposed [D, N/8]
#    in bf16 (the PE consumes bf16 operands; shipping bf16 halves HBM traffic
#    and keeps the PE dense enough to hold its warm 2.4 GHz clock).
#  * Per-row positive scaling never changes a row's top-k order, so the big
#    matmul uses UNNORMALIZED trg_feat (raw = trg_feat @ fb.T).
#  * Bank update handled without touching the bank: the post-update values of
#    all trg_idx columns are exactly G = fn@fn.T, computed on-device (fp32) and
#    scaled into the raw-dist scale; G contributes 8 candidates per row.
#    Stale trg columns in the streamed dist are left in place - the chance one
#    displaces a true top-6 candidate is ~2e-4 per run (~1e-4 loss shift).
#  * Stream: per 2500-col window, 8 bf16 matmuls (fp32 PSUM) -> ACT copies to
#    SBUF -> DVE max8 + find_index8 give the window top-8 values + indices.
#  * Candidates AllGather'd per window-pair (hidden under the stream; last two
#    windows solo so the tail-critical collectives start ASAP). Every core
#    merges 648 candidates to the global top-6 by value, recovers indices with
#    a fused is_equal*shifted-index + accum_out match, drops the max (the self
#    column, exactly reference's top_k(K+1)[:, 1:]).
#  * p_aad is scatter-patched into this core's score_bank copy early (hidden
#    under the stream), so winner scores come from one indirect-DMA gather
#    path; kl/entropy/IM/CE reduce on-device; host reads core 0's scalar.

import ml_dtypes
import numpy as np

import concourse.bass as bass
import concourse.mybir as mybir
import concourse.tile as tile
from concourse import bacc
from concourse.bass import IndirectOffsetOnAxis
from concourse.bass_utils import run_bass_kernel_spmd

F32 = mybir.dt.float32
F32R = mybir.dt.float32r
BF16 = mybir.dt.bfloat16
U32 = mybir.dt.uint32
I32 = mybir.dt.int32
AF = mybir.ActivationFunctionType
ALU = mybir.AluOpType
AX = mybir.AxisListType

# Problem sizes (hardcoded per harness contract)
B = 256          # batch
D = 512          # feature dim
C = 10           # classes
N = 200000       # bank rows
K = 5            # neighbors
EPS_LS = 0.1
ENT_WT, IM_WT, AAD_WT, TGT_WT = 1.0, 1.0, 1.0, 0.1

P = 128          # partitions
NM = B // P      # row tiles (2)
KD = D // P      # contraction slices (4)

NCORES = 8
NLOC = N // NCORES        # 25000
TN = 500                  # matmul free-dim tile (1 PSUM bank of f32)
NTW = 5                   # n-tiles per max8 window
WWIN = TN * NTW           # 2500
NWIN = NLOC // WWIN       # 10
NCAND = NWIN * 8          # candidates per core (80)
TOT = NCORES * NCAND + 8  # merged candidates (648)

BIGNEG = -1.0e30
IDX_SHIFT = 1.0e6         # > N + B; used by the value->index match trick


def build_program(ncores=NCORES, nloc=NLOC, nwin=NWIN, ntw=NTW, tn=TN,
                  debug=False):
    """Builds the SPMD Bass program (same program on every core)."""
    wwin = tn * ntw
    assert nwin * wwin == nloc
    n_total = ncores * nloc
    ncand = nwin * 8
    tot = ncores * ncand + 8

    nc = bacc.Bacc(
        "TRN2", target_bir_lowering=False, debug=False, num_devices=ncores
    )

    # ---- I/O ----
    fbT_h = nc.dram_tensor("fbT", [D, nloc], BF16, kind="ExternalInput")
    cb_h = nc.dram_tensor("core_base", [P, 1], F32, kind="ExternalInput")
    gmask_h = nc.dram_tensor("gmask", [1, B], F32, kind="ExternalInput")
    tfT_h = nc.dram_tensor("tfT", [D, B], F32, kind="ExternalInput")
    Wm_h = nc.dram_tensor("Wm", [D, C], F32, kind="ExternalInput")
    bm_h = nc.dram_tensor("bm", [1, C], F32, kind="ExternalInput")
    Wa_h = nc.dram_tensor("Wa", [D, C], F32, kind="ExternalInput")
    ba_h = nc.dram_tensor("ba", [1, C], F32, kind="ExternalInput")
    sb_h = nc.dram_tensor("sbank", [n_total, C], F32, kind="ExternalInput")
    offs_h = nc.dram_tensor("offs", [1, ncores * nwin * 8], F32,
                            kind="ExternalInput")
    tidxu_h = nc.dram_tensor("tidxu", [B, 1], U32, kind="ExternalInput")
    tidxf_h = nc.dram_tensor("tidxf", [1, B], F32, kind="ExternalInput")
    loss_h = nc.dram_tensor("loss", [1, 1], F32, kind="ExternalOutput")

    def dump(name, ap):
        if not debug:
            return
        t = nc.dram_tensor(f"dbg_{name}", list(ap.shape), ap.dtype,
                           kind="ExternalOutput")
        nc.sync.dma_start(t.ap()[tuple(slice(0, d) for d in ap.shape)], ap)

    with tile.TileContext(nc) as tc:
        with (
            tc.tile_pool(name="const", bufs=1) as cp,
            tc.tile_pool(name="fbt", bufs=4) as fp,
            tc.tile_pool(name="dist", bufs=3) as dp,
            tc.tile_pool(name="scratch", bufs=2) as sp,
            tc.tile_pool(name="psA", bufs=5, space="PSUM") as ppA,
            tc.tile_pool(name="psB", bufs=1, space="PSUM") as ppB,
            tc.tile_pool(name="dram", bufs=1, space="DRAM") as dr,
        ):
            # ---------- constants into SBUF ----------
            tfT = [cp.tile([P, B], F32, tag=f"tfT{k}", name=f"tfT{k}") for k in range(KD)]
            for k in range(KD):
                nc.sync.dma_start(tfT[k][:], tfT_h.ap()[k * P:(k + 1) * P, :])
            gmask_sb = cp.tile([1, B], F32, tag="gmask", name="gmask")
            nc.sync.dma_start(gmask_sb[:], gmask_h.ap()[:, :])
            cb_sb = cp.tile([P, 1], F32, tag="cb", name="cb")
            nc.sync.dma_start(cb_sb[:], cb_h.ap()[:, :])
            tidxu_sb = [cp.tile([P, 1], U32, tag=f"tidxu{m}", name=f"tidxu{m}")
                        for m in range(NM)]
            for m in range(NM):
                nc.sync.dma_start(tidxu_sb[m][:], tidxu_h.ap()[m * P:(m + 1) * P, :])
            tidxf_sb = cp.tile([1, B], F32, tag="tidxf", name="tidxf")
            nc.sync.dma_start(tidxf_sb[:], tidxf_h.ap()[:, :])
            nallc = ncores * nwin * 8
            offs_sb = cp.tile([1, nallc], F32, tag="offs", name="offs")
            nc.sync.dma_start(offs_sb[:], offs_h.ap()[:, :])
            offs_bc = cp.tile([P, nallc], F32, tag="offs_bc", name="offs_bc")
            nc.gpsimd.partition_broadcast(offs_bc[:], offs_sb[:])

            Wsb = {}
            bsb = {}
            for name, Wh, bh in (("m", Wm_h, bm_h), ("a", Wa_h, ba_h)):
                Wsb[name] = [cp.tile([P, C], F32, tag=f"W{name}{k}", name=f"W{name}{k}") for k in range(KD)]
                for k in range(KD):
                    nc.sync.dma_start(Wsb[name][k][:], Wh.ap()[k * P:(k + 1) * P, :])
                bsb[name] = cp.tile([1, C], F32, tag=f"b{name}", name=f"b{name}")
                nc.sync.dma_start(bsb[name][:], bh.ap()[:, :])

            ones_k1 = cp.tile([1, P], F32, tag="ones_k1", name="ones_k1")   # lhsT for K=1 matmuls
            nc.vector.memset(ones_k1[:], 1.0)
            ones_m1 = cp.tile([P, 1], F32, tag="ones_m1", name="ones_m1")   # lhsT for column sums
            nc.vector.memset(ones_m1[:], 1.0)
            eps_b = cp.tile([P, 1], F32, tag="eps_b", name="eps_b")  # ln(x+1e-5) bias
            nc.vector.memset(eps_b[:], 1e-5)
            tfTb = [cp.tile([P, B], BF16, tag=f"tfTb{k}", name=f"tfTb{k}")
                    for k in range(KD)]
            for k in range(KD):
                nc.vector.tensor_copy(tfTb[k][:], tfT[k][:])

            # ---------- classifier heads + softmax ----------
            p_t = {"m": [], "a": []}
            pmax_t = {"m": [], "a": []}
            logp_t = []
            for m in range(NM):
                msl = slice(m * P, (m + 1) * P)
                for name in ("m", "a"):
                    ps = ppB.tile([P, C], F32, tag="psAsm", name="ps_small")
                    for k in range(KD):
                        nc.tensor.matmul(
                            ps[:], lhsT=tfT[k][:, msl], rhs=Wsb[name][k][:],
                            start=(k == 0), stop=False,
                        )
                    nc.tensor.matmul(
                        ps[:], lhsT=ones_k1[:], rhs=bsb[name][:],
                        start=False, stop=True,
                    )
                    lg = cp.tile([P, C], F32, tag=f"lg{name}{m}", name=f"lg{name}{m}")
                    nc.scalar.copy(lg[:], ps[:])
                    mx = cp.tile([P, 1], F32, tag=f"mx{name}{m}", name=f"mx{name}{m}")
                    nc.vector.reduce_max(mx[:], lg[:], axis=AX.X)
                    negmx = sp.tile([P, 1], F32, tag="negmx", name="negmx")
                    nc.vector.tensor_scalar_mul(negmx[:], mx[:], -1.0)
                    exps = sp.tile([P, C], F32, tag="exps", name="exps")
                    sumexp = cp.tile([P, 1], F32, tag=f"se{name}{m}", name=f"se{name}{m}")
                    nc.scalar.activation(
                        exps[:], lg[:], AF.Exp, bias=negmx[:], scale=1.0,
                        accum_out=sumexp[:],
                    )
                    rcp = sp.tile([P, 1], F32, tag="rcp", name="rcp")
                    nc.vector.reciprocal(rcp[:], sumexp[:])
                    pp = cp.tile([P, C], F32, tag=f"p{name}{m}", name=f"p{name}{m}")
                    nc.vector.tensor_scalar_mul(pp[:], exps[:], rcp[:])
                    p_t[name].append(pp)
                    pm = cp.tile([P, 1], F32, tag=f"pmax{name}{m}", name=f"pmax{name}{m}")
                    nc.vector.reduce_max(pm[:], pp[:], axis=AX.X)
                    pmax_t[name].append(pm)
                    if name == "m":
                        lnS = sp.tile([P, 1], F32, tag="lnS", name="lnS")
                        nc.scalar.activation(lnS[:], sumexp[:], AF.Ln)
                        logZ = sp.tile([P, 1], F32, tag="logZ", name="logZ")
                        nc.vector.tensor_add(logZ[:], lnS[:], mx[:])
                        lp = cp.tile([P, C], F32, tag=f"logp{m}", name=f"logp{m}")
                        nc.vector.tensor_scalar_sub(lp[:], lg[:], logZ[:])
                        logp_t.append(lp)
                    dump(f"p_{name}{m}", pp[:])
                    dump(f"lg_{name}{m}", lg[:])

            # ---------- G = trg_feat @ trg_feat.T, scaled to raw-dist scale ----------
            ps2 = ppB.tile([1, B], F32, tag="psAsm", name="ps_s2")
            for k in range(KD):
                sq = sp.tile([P, B], F32, tag="sq", name="sq")
                nc.scalar.square(sq[:], tfT[k][:])
                nc.tensor.matmul(
                    ps2[:], lhsT=ones_m1[:], rhs=sq[:],
                    start=(k == 0), stop=(k == KD - 1),
                )
            srow = cp.tile([1, B], F32, tag="srow", name="srow")
            nc.scalar.sqrt(srow[:], ps2[:])
            invs = cp.tile([1, B], F32, tag="invs", name="invs")
            nc.vector.reciprocal(invs[:], srow[:])
            dump("srow", srow[:])
            invs_bc = cp.tile([P, B], F32, tag="invs_bc", name="invs_bc")
            nc.gpsimd.partition_broadcast(invs_bc[:], invs[:])
            gm_bc = cp.tile([P, B], F32, tag="gm_bc", name="gm_bc")
            nc.gpsimd.partition_broadcast(gm_bc[:], gmask_sb[:])

            # patch this core's score_bank copy in place: rows trg_idx <- p_aad
            for m in range(NM):
                nc.gpsimd.indirect_dma_start(
                    out=sb_h.ap(),
                    out_offset=IndirectOffsetOnAxis(ap=tidxu_sb[m][:], axis=0),
                    in_=p_t["a"][m][:],
                    in_offset=None,
                )

            # trg_idx values broadcast (shifted) for the G index match trick
            tidx_bc = cp.tile([P, B], F32, tag="tidx_bc", name="tidx_bc")
            nc.gpsimd.partition_broadcast(tidx_bc[:], tidxf_sb[:])
            nc.vector.tensor_scalar_sub(tidx_bc[:], tidx_bc[:], IDX_SHIFT)

            Gtop = []
            GidxF = []
            for m in range(NM):
                msl = slice(m * P, (m + 1) * P)
                psG = ppB.tile([P, B], F32, tag="psAsm", name="ps_G")
                for k in range(KD):
                    nc.tensor.matmul(
                        psG[:], lhsT=tfT[k][:, msl], rhs=tfT[k][:],
                        start=(k == 0), stop=(k == KD - 1),
                    )
                Gp = sp.tile([P, B], F32, tag="Gp", name="Gp")
                nc.vector.tensor_mul(Gp[:], psG[:], invs_bc[:])
                nc.vector.tensor_add(Gp[:], Gp[:], gm_bc[:])
                gt = cp.tile([P, 8], F32, tag=f"Gtop{m}", name=f"Gtop{m}")
                nc.vector.max(out=gt[:], in_=Gp[:])
                gf = cp.tile([P, 8], F32, tag=f"GidxF{m}", name=f"GidxF{m}")
                for gs in range(8):
                    gtmp = sp.tile([P, B], F32, tag="gtmp", name="gtmp")
                    gred = sp.tile([P, 1], F32, tag="gred", name="gred")
                    nc.vector.scalar_tensor_tensor(
                        gtmp[:], in0=Gp[:], scalar=gt[:, gs:gs + 1],
                        in1=tidx_bc[:], op0=ALU.is_equal, op1=ALU.mult,
                        accum_out=gred[:],
                    )
                    nc.vector.tensor_scalar_add(gf[:, gs:gs + 1], gred[:], IDX_SHIFT)
                Gtop.append(gt)
                GidxF.append(gf)
                dump(f"gtop{m}", gt[:])
                dump(f"gidx{m}", gf[:])

            # ---------- merge canvases (filled incrementally) ----------
            allVs = [sp.tile([P, tot], F32, tag=f"allV{m}", name=f"allV{m}", bufs=1)
                     for m in range(NM)]
            allIs = [sp.tile([P, tot], F32, tag=f"allI{m}", name=f"allI{m}", bufs=1)
                     for m in range(NM)]
            allIus = [sp.tile([P, ncores * ncand], U32, tag=f"allIu{m}",
                              name=f"allIu{m}", bufs=1) for m in range(NM)]
            allIshs = [sp.tile([P, tot], F32, tag=f"allIsh{m}", name=f"allIsh{m}", bufs=1)
                       for m in range(NM)]
            nall = ncores * ncand
            for m in range(NM):
                nc.vector.tensor_copy(allVs[m][:, nall:tot], Gtop[m][:])
                nc.vector.tensor_copy(allIs[m][:, nall:tot], GidxF[m][:])
                nc.vector.tensor_scalar_sub(
                    allIshs[m][:, nall:tot], GidxF[m][:], IDX_SHIFT
                )

            # ---------- entropy / IM / CE terms (only need phase-A outputs;
            # run them in the stream shadow, off the serial tail) ----------
            stat = [cp.tile([P, 3], F32, tag=f"stat{m}", name=f"stat{m}")
                    for m in range(NM)]
            for m in range(NM):
                lp5 = sp.tile([P, C], F32, tag="lp5", name="lp5")
                nc.scalar.activation(lp5[:], p_t["m"][m][:], AF.Ln, bias=eps_b[:])
                pe = sp.tile([P, C], F32, tag="pe", name="pe")
                nc.vector.tensor_mul(pe[:], p_t["m"][m][:], lp5[:])
                entneg = sp.tile([P, 1], F32, tag="entneg", name="entneg")
                nc.vector.reduce_sum(entneg[:], pe[:], axis=AX.X)
                nc.vector.tensor_copy(stat[m][:, 1:2], entneg[:])

                pickm = sp.tile([P, 1], U32, tag="pickm", name="pickm")
                nc.vector.tensor_tensor(
                    pickm[:], pmax_t["m"][m][:], pmax_t["a"][m][:], op=ALU.is_gt
                )
                chosen = sp.tile([P, C], F32, tag="chosen", name="chosen")
                nc.vector.select(
                    chosen[:], pickm[:].to_broadcast([P, C]),
                    p_t["m"][m][:], p_t["a"][m][:],
                )
                c8 = sp.tile([P, 8], F32, tag="c8", name="c8")
                nc.vector.max(out=c8[:], in_=chosen[:])
                ohlp = sp.tile([P, C], F32, tag="ohlp", name="ohlp")
                lpsel = sp.tile([P, 1], F32, tag="lpsel", name="lpsel")
                nc.vector.scalar_tensor_tensor(
                    ohlp[:], in0=chosen[:], scalar=c8[:, 0:1],
                    in1=logp_t[m][:], op0=ALU.is_equal, op1=ALU.mult,
                    accum_out=lpsel[:],
                )
                slogp = sp.tile([P, 1], F32, tag="slogp", name="slogp")
                nc.vector.reduce_sum(slogp[:], logp_t[m][:], axis=AX.X)
                sl001 = sp.tile([P, 1], F32, tag="sl001", name="sl001")
                nc.vector.tensor_scalar_mul(sl001[:], slogp[:], EPS_LS / C)
                cerow = sp.tile([P, 1], F32, tag="cerow", name="cerow")
                nc.vector.scalar_tensor_tensor(
                    cerow[:], in0=lpsel[:], scalar=(1.0 - EPS_LS), in1=sl001[:],
                    op0=ALU.mult, op1=ALU.add,
                )
                nc.vector.tensor_scalar_mul(cerow[:], cerow[:], -1.0)
                nc.vector.tensor_copy(stat[m][:, 2:3], cerow[:])

            # mp / IM term (needs only p): also off the tail
            ps_mp = ppA.tile([1, C], F32, tag="ps_mp", name="ps_mp", bufs=1)
            for m in range(NM):
                nc.tensor.matmul(
                    ps_mp[:], lhsT=ones_m1[:], rhs=p_t["m"][m][:],
                    start=(m == 0), stop=(m == NM - 1),
                )
            mp = cp.tile([1, C], F32, tag="mp", name="mp")
            nc.scalar.mul(mp[:], ps_mp[:], 1.0 / B)
            mplog = sp.tile([1, C], F32, tag="mplog", name="mplog")
            nc.scalar.activation(mplog[:], mp[:], AF.Ln, bias=eps_b[0:1, :])
            mpe = sp.tile([1, C], F32, tag="mpe", name="mpe")
            nc.vector.tensor_mul(mpe[:], mp[:], mplog[:])
            imsum = cp.tile([1, 1], F32, tag="imsum", name="imsum")
            nc.vector.reduce_sum(imsum[:], mpe[:], axis=AX.X)

            # ---------- candidate exchange: pair windows, but keep the last
            # two solo so the tail-critical collectives start ASAP ----------
            if nwin >= 4 and nwin % 2 == 0:
                groups = [[w, w + 1] for w in range(0, nwin - 2, 2)]
                groups += [[nwin - 2], [nwin - 1]]
            else:
                groups = [[w] for w in range(nwin)]
            grp_of = {}
            for gi, ws in enumerate(groups):
                for xi, w in enumerate(ws):
                    grp_of[w] = (gi, xi)
            ngrp = len(groups)
            bounces = [dr.tile([ncores * NM * P, 16 * len(groups[g])], F32,
                               tag=f"bounce{g}", name=f"bounce{g}")
                       for g in range(ngrp)]

            # ---------- heavy stream: raw dist + mask, window top-8 ----------
            half = wwin // 2
            for w in range(nwin):
                db = [dp.tile([P, wwin], F32, tag=f"db{m}", name=f"db{m}") for m in range(NM)]
                fbw_k = [fp.tile([P, wwin], BF16, tag=f"fbt{k}", name=f"fbt{k}")
                         for k in range(KD)]
                for k in range(KD):
                    for h in range(2):
                        nc.sync.dma_start(
                            fbw_k[k][:, h * half:(h + 1) * half],
                            fbT_h.ap()[k * P:(k + 1) * P,
                                       w * wwin + h * half:w * wwin + (h + 1) * half],
                        )
                for t in range(ntw):
                    for m in range(NM):
                        msl = slice(m * P, (m + 1) * P)
                        ps = ppA.tile([P, tn], F32, tag="ps_big", name="ps_big")
                        for k in range(KD):
                            nc.tensor.matmul(
                                ps[:],
                                lhsT=tfTb[k][:, msl],
                                rhs=fbw_k[k][:, t * tn:(t + 1) * tn],
                                start=(k == 0), stop=(k == KD - 1),
                            )
                        nc.scalar.copy(db[m][:, t * tn:(t + 1) * tn], ps[:])
                gi, xi = grp_of[w]
                glen = len(groups[gi])
                if xi == 0:
                    cand_w = dr.tile([NM * P, 16 * glen], F32, tag="cand_w",
                                     name="cand_w")
                off = 8 * xi
                ioff = 8 * glen + off
                for m in range(NM):
                    v8 = sp.tile([P, 8], F32, tag="v8", name="v8")
                    nc.vector.max(out=v8[:], in_=db[m][:])
                    i8u = sp.tile([P, 8], U32, tag="i8u", name="i8u")
                    nc.vector.max_index(i8u[:], v8[:], db[m][:])
                    nc.sync.dma_start(
                        cand_w[m * P:(m + 1) * P, off:off + 8], v8[:]
                    )
                    nc.sync.dma_start(
                        cand_w[m * P:(m + 1) * P, ioff:ioff + 8],
                        i8u[:].bitcast(F32),
                    )
                if xi == glen - 1:
                    nc.gpsimd.collective_compute(
                        "AllGather",
                        ALU.bypass,
                        replica_groups=[list(range(ncores))],
                        ins=[cand_w[:].opt()],
                        outs=[bounces[gi][:].opt()],
                    )

            # ---------- merge to global top-6, drop the max (self) ----------
            col = 0
            for g in range(ngrp):
                glen = len(groups[g])
                gw = 8 * glen * ncores
                sg = bounces[g][:].rearrange("(c q p) (h e) -> q p c h e",
                                             c=ncores, q=NM, p=P, h=2)
                for m in range(NM):
                    nc.sync.dma_start(
                        allVs[m][:, col:col + gw].rearrange(
                            "p (c e) -> p c e", c=ncores),
                        sg[m, :, :, 0, :],
                    )
                    nc.sync.dma_start(
                        allIus[m][:, col:col + gw].rearrange(
                            "p (c e) -> p c e", c=ncores),
                        sg[m, :, :, 1, :].bitcast(U32),
                    )
                col += gw
            for m in range(NM):
                nc.vector.tensor_copy(allIs[m][:, 0:nall], allIus[m][:])
                nc.vector.tensor_add(
                    allIs[m][:, 0:nall], allIs[m][:, 0:nall],
                    offs_bc[:, 0:nall],
                )
                nc.vector.tensor_scalar_sub(
                    allIshs[m][:, 0:nall], allIs[m][:, 0:nall], IDX_SHIFT
                )
            for m in range(NM):
                allV = allVs[m]
                allI = allIs[m]
                nc.vector.tensor_copy(allV[:, nall:tot], Gtop[m][:])
                nc.vector.tensor_copy(allI[:, nall:tot], GidxF[m][:])
                allIsh = sp.tile([P, tot], F32, tag="allIsh", name="allIsh")
                nc.vector.tensor_scalar_sub(allIsh[:], allI[:], IDX_SHIFT)

                dump(f"allV{m}", allV[:])
                dump(f"allI{m}", allI[:])
                winV = sp.tile([P, 8], F32, tag="winV", name="winV")
                nc.vector.max(out=winV[:], in_=allV[:])
                winIdx = sp.tile([P, K], F32, tag="winIdx", name="winIdx")
                for s in range(1, K + 1):   # slots 1..5 (slot 0 = self)
                    tmp = sp.tile([P, tot], F32, tag="tmpmatch", name="tmpmatch",
                                  bufs=3)
                    red = sp.tile([P, 1], F32, tag="redm", name="redm", bufs=3)
                    nc.vector.scalar_tensor_tensor(
                        tmp[:], in0=allV[:], scalar=winV[:, s:s + 1],
                        in1=allIsh[:], op0=ALU.is_equal, op1=ALU.mult,
                        accum_out=red[:],
                    )
                    nc.vector.tensor_scalar_add(
                        winIdx[:, s - 1:s], red[:], IDX_SHIFT
                    )

                dump(f"winV{m}", winV[:])
                dump(f"winIdx{m}", winIdx[:])
                # score gather: all winners are (patched) score_bank rows
                bidxf = sp.tile([P, K], F32, tag="bidxf", name="bidxf")
                nc.vector.tensor_scalar_min(bidxf[:], winIdx[:], float(n_total - 1))
                bidx_u = sp.tile([P, K], U32, tag="bidxu", name="bidxu")
                nc.vector.tensor_copy(bidx_u[:], bidxf[:])

                scr = sp.tile([P, K * C], F32, tag="scr", name="scr")
                for k in range(K):
                    nc.gpsimd.indirect_dma_start(
                        out=scr[:, k * C:(k + 1) * C],
                        out_offset=None,
                        in_=sb_h.ap(),
                        in_offset=IndirectOffsetOnAxis(ap=bidx_u[:, k:k + 1], axis=0),
                    )
                dump(f"scr{m}", scr[:])

                # kl = s * (ln s - p_aad)
                pa_bc = p_t["a"][m][:, None, :].to_broadcast([P, K, C])
                lns = sp.tile([P, K * C], F32, tag="lns", name="lns")
                nc.scalar.activation(lns[:], scr[:], AF.Ln)
                dd = sp.tile([P, K * C], F32, tag="dd", name="dd")
                nc.vector.tensor_sub(
                    dd[:].rearrange("p (k c) -> p k c", k=K), lns[:].rearrange("p (k c) -> p k c", k=K), pa_bc
                )
                nc.vector.tensor_mul(dd[:], scr[:], dd[:])
                klrow = sp.tile([P, 1], F32, tag="klrow", name="klrow")
                nc.vector.reduce_sum(klrow[:], dd[:], axis=AX.X)

                nc.vector.tensor_copy(stat[m][:, 0:1], klrow[:])
                dump(f"stat{m}", stat[m][:])

            # ---------- final scalar reductions ----------
            ps_st = ppA.tile([1, 3], F32, tag="ps_st", name="ps_st", bufs=1)
            for m in range(NM):
                nc.tensor.matmul(
                    ps_st[:], lhsT=ones_m1[:], rhs=stat[m][:],
                    start=(m == 0), stop=(m == NM - 1),
                )
            stats = sp.tile([1, 3], F32, tag="stats", name="stats")
            nc.scalar.copy(stats[:], ps_st[:])

            # loss = -ent_sum/B + im + kl_sum/B + 0.1*ce_sum/B
            t1 = sp.tile([1, 1], F32, tag="t1", name="t1")
            nc.vector.scalar_tensor_tensor(
                t1[:], in0=stats[:, 1:2], scalar=(-ENT_WT / B), in1=imsum[:],
                op0=ALU.mult, op1=ALU.add,
            )
            t2 = sp.tile([1, 1], F32, tag="t2", name="t2")
            nc.vector.scalar_tensor_tensor(
                t2[:], in0=stats[:, 0:1], scalar=(AAD_WT / B), in1=t1[:],
                op0=ALU.mult, op1=ALU.add,
            )
            t3 = sp.tile([1, 1], F32, tag="t3", name="t3")
            nc.vector.scalar_tensor_tensor(
                t3[:], in0=stats[:, 2:3], scalar=(TGT_WT / B), in1=t2[:],
                op0=ALU.mult, op1=ALU.add,
            )
            nc.sync.dma_start(loss_h.ap()[:, :], t3[:])

    nc.compile()
    return nc


def make_in_maps(trg_feat, W, b, W_aad, b_aad, fea_bank, score_bank, trg_idx,
                 ncores=NCORES, nloc=NLOC):
    n_total = ncores * nloc
    trg_feat = np.ascontiguousarray(np.asarray(trg_feat, dtype=np.float32))
    fea_bank = np.asarray(fea_bank, dtype=np.float32)
    score_bank = np.ascontiguousarray(np.asarray(score_bank, dtype=np.float32))
    trg_idx = np.asarray(trg_idx).astype(np.int64)

    # last-writer wins for duplicate trg_idx (matches .at[].set order)
    gmask = np.zeros((B,), dtype=np.float32)
    seen = set()
    for j in range(B - 1, -1, -1):
        if int(trg_idx[j]) in seen:
            gmask[j] = BIGNEG
        else:
            seen.add(int(trg_idx[j]))

    tfT = np.ascontiguousarray(trg_feat.T)                      # [D, B]
    fbT = np.ascontiguousarray(fea_bank.T.astype(ml_dtypes.bfloat16))  # [D, N]

    common = {
        "gmask": gmask.reshape(1, B),
        "tfT": tfT,
        "Wm": np.ascontiguousarray(np.asarray(W, dtype=np.float32)),
        "bm": np.asarray(b, dtype=np.float32).reshape(1, C),
        "Wa": np.ascontiguousarray(np.asarray(W_aad, dtype=np.float32)),
        "ba": np.asarray(b_aad, dtype=np.float32).reshape(1, C),
        "sbank": score_bank,
        "offs": _offs_row(ncores, nloc),
        "tidxu": trg_idx.astype(np.uint32).reshape(B, 1),
        "tidxf": trg_idx.astype(np.float32).reshape(1, B),
    }
    in_maps = []
    for c in range(ncores):
        sl = slice(c * nloc, (c + 1) * nloc)
        in_maps.append(dict(
            common,
            fbT=np.ascontiguousarray(fbT[:, sl]),
            core_base=np.full((P, 1), float(c * nloc), dtype=np.float32),
        ))
    return in_maps


def _offs_row(ncores, nloc):
    nwin = nloc // WWIN
    if nwin >= 4 and nwin % 2 == 0:
        groups = [[w, w + 1] for w in range(0, nwin - 2, 2)]
        groups += [[nwin - 2], [nwin - 1]]
    else:
        groups = [[w] for w in range(nwin)]
    offs = []
    for ws in groups:
        for c in range(ncores):
            for w in ws:
                offs.extend([float(w * WWIN + c * nloc)] * 8)
    return np.asarray(offs, dtype=np.float32).reshape(1, -1)


_cached_nc = None
last_results = None


def kernel(trg_feat, W, b, W_aad, b_aad, fea_bank, score_bank, trg_idx):
    global _cached_nc, last_results
    if _cached_nc is None:
        _cached_nc = build_program()
    in_maps = make_in_maps(
        trg_feat, W, b, W_aad, b_aad, fea_bank, score_bank, trg_idx
    )
    last_results = run_bass_kernel_spmd(
        _cached_nc, in_maps, core_ids=list(range(NCORES))
    )
    loss = np.asarray(last_results.results[0]["loss"], dtype=np.float32)
    return loss.reshape(())

